# revision 8
# baseline (speedup 1.0000x reference)
"""LoRA linear layer (out = x @ (W + s*A@B) + bias) on 8 Trainium2 NeuronCores.

Sharding: data-parallel over rows of x (M = 4*2048 = 8192 -> 1024 rows/core);
each core computes its row-slice against the full weight matrix. The x slice
is supplied in [K, M] layout (pure layout transform done while sharding) so
the contraction dim lands on SBUF partitions.

Per-core kernel (all fp32r = fp32 storage, ~fp22 multiply, fp32 accumulate):
  - stationary = W tile [128k x 128n], moving = xT [128k x 512m]; 32 K-tile
    matmuls accumulate each [128n x 512m] PSUM tile (out is computed
    transposed; the host transposes it back)
  - LoRA: xAT = A^T @ xT (rank 16) computed once on device; a 33rd rank-16
    matmul per PSUM tile adds (xA @ sB)^T into the same accumulation
  - bias is added during the PSUM -> SBUF copy on the scalar engine
    (per-partition bias = per-output-channel in the transposed layout)
"""
import numpy as np

import concourse.bass as bass
import concourse.tile as tile
from concourse import bacc, mybir
from concourse.bass_utils import run_bass_kernel_spmd

P = 128
N_CORES = 8
BATCH, SEQ = 4, 2048
D_IN, D_OUT, RANK = 4096, 4096, 16
M_FULL = BATCH * SEQ          # 8192
M_C = M_FULL // N_CORES       # 1024 rows per core
KT = D_IN // P                # 32 k-tiles
MC = M_C // 512               # 2 moving chunks of 512
NTP = D_OUT // 256            # 16 n-tile-pairs (W loaded 256 cols at a time)
F32 = mybir.dt.float32
F32R = mybir.dt.float32r

_NC_CACHE = None


def _build_nc():
    nc = bacc.Bacc("TRN2", target_bir_lowering=False, debug=False,
                   num_devices=N_CORES)
    xt_d = nc.dram_tensor("xt", [D_IN, M_C], F32, kind="ExternalInput").ap()
    w_d = nc.dram_tensor("w", [D_IN, D_OUT], F32, kind="ExternalInput").ap()
    bias_d = nc.dram_tensor("bias", [D_OUT], F32, kind="ExternalInput").ap()
    a_d = nc.dram_tensor("lora_a", [D_IN, RANK], F32, kind="ExternalInput").ap()
    b_d = nc.dram_tensor("lora_b", [RANK, D_OUT], F32, kind="ExternalInput").ap()
    outt_d = nc.dram_tensor("outt", [D_OUT, M_C], F32, kind="ExternalOutput").ap()

    with tile.TileContext(nc) as tc:
        with (
            tc.tile_pool(name="singles", bufs=1) as singles,
            tc.tile_pool(name="wts", bufs=40) as w_pool,
            tc.tile_pool(name="bt", bufs=3) as b_pool,
            tc.tile_pool(name="outs", bufs=4) as out_pool,
            tc.tile_pool(name="psum", bufs=8, space="PSUM") as psum_pool,
        ):

            # Fused first sweep (kt-outer): stream xT + the first W column
            # block together, computing xAT (rank-16) and the first n-tile
            # pair in one pass so the PE is never starved while the 16 MB
            # x slice lands.
            xT = singles.tile([P, KT, M_C], F32R)
            xat = singles.tile([RANK, M_C], F32R)
            xps = [psum_pool.tile([RANK, 512], F32, tag="ps",
                                  name=f"xp_{mc}") for mc in range(MC)]
            ps0 = {(sub, mc): psum_pool.tile([P, 512], F32, tag="ps",
                                             name=f"ps0_{sub}_{mc}")
                   for sub in range(2) for mc in range(MC)}
            w0_tiles = []
            # first k-tile's inputs land before anything else so the PE can
            # start immediately; the strided lora_A load follows them
            nc.sync.dma_start(out=xT[:, 0, :],
                              in_=xt_d[0:P, :].bitcast(F32R))
            wt0 = w_pool.tile([P, 256], F32R, tag="wt", name="w0_0")
            nc.sync.dma_start(out=wt0, in_=w_d[0:P, 0:256].bitcast(F32R))
            a_sb = singles.tile([P, KT, RANK], F32R)
            nc.sync.dma_start(
                out=a_sb,
                in_=a_d.bitcast(F32R).rearrange("(kt p) r -> p kt r", p=P),
            )
            for kt in range(KT):
                if kt == 0:
                    wt = wt0
                else:
                    nc.sync.dma_start(
                        out=xT[:, kt, :],
                        in_=xt_d[kt * P:(kt + 1) * P, :].bitcast(F32R),
                    )
                    wt = w_pool.tile([P, 256], F32R, tag="wt", name=f"w0_{kt}")
                    nc.sync.dma_start(
                        out=wt,
                        in_=w_d[kt * P:(kt + 1) * P, 0:256].bitcast(F32R))
                w0_tiles.append(wt)
                for sub in range(2):
                    for mc in range(MC):
                        nc.tensor.matmul(
                            ps0[(sub, mc)],
                            wt[:, sub * P:(sub + 1) * P],
                            xT[:, kt, mc * 512:(mc + 1) * 512],
                            start=(kt == 0),
                            stop=False,
                        )
                for mc in range(MC):
                    nc.tensor.matmul(
                        xps[mc],
                        a_sb[:, kt, :],
                        xT[:, kt, mc * 512:(mc + 1) * 512],
                        start=(kt == 0),
                        stop=(kt == KT - 1),
                    )
            # per-output-channel bias striped so channel lands on partition:
            # bias_cols[p, nt] = bias[nt*128 + p]
            bias_cols = singles.tile([P, D_OUT // P], F32)
            nc.sync.dma_start(
                out=bias_cols, in_=bias_d.rearrange("(nt p) -> p nt", p=P))
            for mc in range(MC):
                nc.vector.tensor_copy(
                    out=xat[:, mc * 512:(mc + 1) * 512], in_=xps[mc])
            bt0 = b_pool.tile([RANK, 256], F32R, tag="bt", name="bt0")
            nc.sync.dma_start(out=bt0, in_=b_d[:, 0:256].bitcast(F32R))
            for sub in range(2):
                nt = sub
                for mc in range(MC):
                    nc.tensor.matmul(
                        ps0[(sub, mc)],
                        bt0[:, sub * P:(sub + 1) * P],
                        xat[:, mc * 512:(mc + 1) * 512],
                        start=False,
                        stop=True,
                    )
                    ob = out_pool.tile([P, 512], F32, tag="ob",
                                       name=f"ob0_{sub}_{mc}")
                    nc.scalar.activation(
                        ob, ps0[(sub, mc)],
                        mybir.ActivationFunctionType.Identity,
                        bias=bias_cols[:, nt:nt + 1],
                    )
                    nc.sync.dma_start(
                        out=outt_d[nt * P:(nt + 1) * P,
                                   mc * 512:(mc + 1) * 512],
                        in_=ob,
                    )

            # main loop: out^T[n, m] accumulated per [128n x 512m] PSUM tile
            for ntp in range(1, NTP):
                nsl = slice(ntp * 256, (ntp + 1) * 256)
                w_tiles = []
                for kt in range(KT):
                    wt = w_pool.tile([P, 256], F32R)
                    nc.sync.dma_start(
                        out=wt,
                        in_=w_d[kt * P:(kt + 1) * P, nsl].bitcast(F32R),
                    )
                    w_tiles.append(wt)
                bt = b_pool.tile([RANK, 256], F32R)
                nc.sync.dma_start(out=bt, in_=b_d[:, nsl].bitcast(F32R))

                for sub in range(2):
                    nt = ntp * 2 + sub
                    psums = [psum_pool.tile([P, 512], F32, tag="ps",
                                            name=f"ps_{nt}_{mc}")
                             for mc in range(MC)]
                    for kt in range(KT):
                        for mc in range(MC):
                            nc.tensor.matmul(
                                psums[mc],
                                w_tiles[kt][:, sub * P:(sub + 1) * P],
                                xT[:, kt, mc * 512:(mc + 1) * 512],
                                start=(kt == 0),
                                stop=False,
                            )
                    for mc in range(MC):
                        nc.tensor.matmul(
                            psums[mc],
                            bt[:, sub * P:(sub + 1) * P],
                            xat[:, mc * 512:(mc + 1) * 512],
                            start=False,
                            stop=True,
                        )
                        ob = out_pool.tile([P, 512], F32)
                        nc.scalar.activation(
                            ob, psums[mc],
                            mybir.ActivationFunctionType.Identity,
                            bias=bias_cols[:, nt:nt + 1],
                        )
                        nc.sync.dma_start(
                            out=outt_d[nt * P:(nt + 1) * P,
                                       mc * 512:(mc + 1) * 512],
                            in_=ob,
                        )

    nc.compile()
    return nc


def get_nc():
    global _NC_CACHE
    if _NC_CACHE is None:
        _NC_CACHE = _build_nc()
    return _NC_CACHE


def make_in_maps(x, W, bias, lora_A, lora_B, scaling):
    x2 = np.asarray(x, dtype=np.float32).reshape(M_FULL, D_IN)
    w = np.ascontiguousarray(np.asarray(W, dtype=np.float32))
    b = np.ascontiguousarray(np.asarray(bias, dtype=np.float32))
    a = np.ascontiguousarray(np.asarray(lora_A, dtype=np.float32))
    s = np.float32(np.asarray(scaling).astype(np.float64))
    bs = np.ascontiguousarray(s * np.asarray(lora_B, dtype=np.float32))
    return [
        {
            "xt": np.ascontiguousarray(x2[c * M_C:(c + 1) * M_C].T),
            "w": w,
            "bias": b,
            "lora_a": a,
            "lora_b": bs,
        }
        for c in range(N_CORES)
    ]


def assemble_output(results):
    """results: list of per-core dicts with 'outt' [D_OUT, M_C]."""
    out = np.concatenate(
        [results[c]["outt"].T for c in range(N_CORES)], axis=0)
    return np.ascontiguousarray(out).reshape(BATCH, SEQ, D_OUT)


def kernel(x, W, bias, lora_A, lora_B, scaling):
    nc = get_nc()
    in_maps = make_in_maps(x, W, bias, lora_A, lora_B, scaling)
    res = run_bass_kernel_spmd(nc, in_maps, core_ids=list(range(N_CORES)))
    return assemble_output(res.results)
